# revision 1
# baseline (speedup 1.0000x reference)
"""Trainium2 Bass kernel for the dual-branch spatial-reduction attention module.

Sharding (8 NeuronCores): branch-parallel (cores 0-3 -> branch 0, cores 4-7 ->
branch 1); within a branch quad, query-token-parallel for the attention (each
core owns 1024 of the 4096 query tokens). The spatial-reduction conv +
LayerNorm + k/v projections over the 1024 kv positions are computed fully on
every core: replicating that work (~52k extra PE rows, ~22us) is cheaper than
the cross-core AllGather it replaces (15us constant + 26us transfer on the
collective device, which stalled the PE for ~53us in the previous version),
and it removes all cross-core synchronization. The host does layout prep
(transposes, bf16 casts, weight folding, conv patch gathering) and the final
TokenExchange between branches.

Per-core program (all matmuls bf16 with fp32 PSUM accumulation):
  - conv = 4 shifted matmuls over host-pregathered 2x2 patches (token-major,
    8 chunks of 128 positions, input DMA split per chunk so the conv streams
    behind the transfer) + bias via a K=1 ones-matmul; LayerNorm fused on the
    PSUM output (bn_stats/bn_aggr, vector reciprocal); gamma/beta folded into
    Wkv on the host; xn transposed to channel-major via bf16 DMA-XBAR tiles
  - the k-projection bias is dropped entirely (it only adds a per-(query,
    head) constant to the logits, which softmax cancels) and the v-projection
    bias is folded into the output-projection bias (bp + bv @ Wp), since
    softmax weights sum to one
  - q proj channel-major (softmax scale folded into Wq); k proj channel-major
    / v proj token-major; PSUM->SBUF copies on the DVE; independent PE work
    (qproj/kproj/vproj/early qk) is interleaved between conv chunks so the
    PE never restarts from idle at a dependency release (the cost model's
    p-state ramp punishes idle restarts)
  - attention: qk for a head pair packed into PE row groups 0-63/64-127
    (K=64 each); softmax exp without max subtraction (logits are tiny by
    construction) on the Activation engine, with qk tiles issued one mc
    chunk ahead of their attn@v consumer and 16 tiles pre-issued during the
    conv tail so the exp pipeline never starves; v gets an appended ones
    column so the softmax denominator falls out of the attn@v matmul (row 64)
  - per-(head, n) normalization: DVE stages av out of PSUM (freeing the
    bank for the next pair's attn@v early) and takes the reciprocal of the
    denominator row; gpsimd broadcasts it and applies the multiply
  - out proj token-major per n-chunk right after its attention completes;
    bias via a K=1 ones-matmul; mask applied via per-partition
    tensor_scalar_mul; output stores on the SP DMA queue
"""

import sys

sys.path.insert(0, "/opt/trn_rl_repo")

import numpy as np
import ml_dtypes

BF16 = ml_dtypes.bfloat16

NUM_HEADS = 8
SR = 2
LN_EPS = 1e-5
MASK_THRESHOLD = 0.02
B, N, C = 1, 4096, 512
H = W = 64
M = N // (SR * SR)  # 1024 kv positions
HD = C // NUM_HEADS  # 64
NQ = N // 4  # 1024 query tokens per core
N_CORES = 8

_compiled = None


def _build():
    import concourse.bass as bass
    import concourse.tile as tile
    from concourse import bacc, mybir

    f32 = mybir.dt.float32
    bf16 = mybir.dt.bfloat16
    fp8 = mybir.dt.float8e4

    nc = bacc.Bacc("TRN2", target_bir_lowering=False, debug=False,
                   num_devices=N_CORES)

    # ---- DRAM I/O ----
    xp_d = nc.dram_tensor("xp", [4, C, M], bf16, kind="ExternalInput")
    xq_d = nc.dram_tensor("xqT", [C, NQ], bf16, kind="ExternalInput")
    wq_d = nc.dram_tensor("wq", [C, C], bf16, kind="ExternalInput")
    wsr_d = nc.dram_tensor("wsr", [4, C, C], bf16, kind="ExternalInput")
    bsr_d = nc.dram_tensor("bsr_r", [1, C], bf16, kind="ExternalInput")
    wk_d = nc.dram_tensor("wk", [C, C], bf16, kind="ExternalInput")
    wv_d = nc.dram_tensor("wv", [C, C], bf16, kind="ExternalInput")
    wp_d = nc.dram_tensor("wp", [C, C], bf16, kind="ExternalInput")
    bp_d = nc.dram_tensor("bp_r", [1, C], bf16, kind="ExternalInput")
    mask_d = nc.dram_tensor("mask_s", [128, 8], f32, kind="ExternalInput")
    out_d = nc.dram_tensor("out", [NQ, C], bf16, kind="ExternalOutput")

    P = 128
    CC = C // P  # 4 channel chunks
    MC = M // P  # 8 kv-position chunks
    N2 = NQ // 512  # 2 query free-dim chunks
    HP = NUM_HEADS // 2  # 4 head pairs
    DVE_MC = ()  # DVE has no exp/pow opcode on NeuronCore v3; all exp on ScalarE

    from contextlib import ExitStack
    with tile.TileContext(nc) as tc, ExitStack() as ctx:
        consts = ctx.enter_context(tc.tile_pool(name="consts", bufs=1))
        psA = ctx.enter_context(tc.tile_pool(name="psA", bufs=2, space="PSUM"))
        psQK = ctx.enter_context(tc.tile_pool(name="psQK", bufs=2, space="PSUM"))
        psAV = ctx.enter_context(tc.tile_pool(name="psAV", bufs=2, space="PSUM"))
        ptp = ctx.enter_context(tc.tile_pool(name="ptp", bufs=21))
        xnp = ctx.enter_context(tc.tile_pool(name="xnp", bufs=4))
        stats = ctx.enter_context(tc.tile_pool(name="stats", bufs=2))
        outs = ctx.enter_context(tc.tile_pool(name="outs", bufs=4))

        def load_cpn(dst, src, eng=None):
            (eng or nc.sync).dma_start(
                out=dst, in_=src.rearrange("(cc p) n -> p cc n", p=P))

        # ---- input DMAs: one sync-queue stream interleaving the conv's
        # streamed inputs with the attention weights in consumption order,
        # so the shared DMA engines feed the conv just ahead of the PE
        # without weight traffic starving it ----
        bsr_sb = consts.tile([1, C], bf16)
        nc.sync.dma_start(out=bsr_sb, in_=bsr_d.ap())
        wsr_sb = consts.tile([P, 4, CC, C], bf16)
        for ij in range(4):
            nc.sync.dma_start(
                out=wsr_sb[:, ij],
                in_=wsr_d.ap()[ij].rearrange("(cc p) n -> p cc n", p=P))
        # xp in 4 big chunks (256 m-columns = 512B per-partition runs, full
        # DMA rate), with wq/xq slotted after the first so qproj can start
        # while later conv inputs stream
        xp_sb = consts.tile([P, 4, CC, M], bf16)
        wq_sb = consts.tile([P, CC, C], bf16)
        xq_sb = consts.tile([P, CC, NQ], bf16)
        wk_sb = consts.tile([P, CC, C], bf16)
        wv_sb = consts.tile([P, CC, C], bf16)
        wp_sb = consts.tile([P, CC, C], bf16)

        def xp_load(i):
            nc.sync.dma_start(
                out=xp_sb[:, :, :, i * 256:(i + 1) * 256],
                in_=xp_d.ap()[:, :, i * 256:(i + 1) * 256].rearrange(
                    "ij (cc p) n -> p ij cc n", p=P))

        xp_load(0)
        load_cpn(wq_sb, wq_d)
        load_cpn(xq_sb, xq_d)
        xp_load(1)
        xp_load(2)
        xp_load(3)
        load_cpn(wk_sb, wk_d)
        load_cpn(wv_sb, wv_d)
        load_cpn(wp_sb, wp_d)
        bp_sb = consts.tile([1, C], bf16)
        nc.sync.dma_start(out=bp_sb, in_=bp_d.ap())
        mask_sb = consts.tile([P, 8], f32)
        nc.sync.dma_start(out=mask_sb, in_=mask_d.ap())

        ones128 = consts.tile([1, P], bf16)
        nc.vector.memset(ones128, 1.0)
        ones512 = consts.tile([1, C], bf16)
        nc.vector.memset(ones512, 1.0)
        eps_sb = consts.tile([P, 1], f32)
        nc.vector.memset(eps_sb, LN_EPS)

        # PE warm-up: dummy matmuls during the input-DMA wait so the HAM
        # clock gate is released before the conv starts
        warm_ps = psA.tile([1, 512], f32, tag="psA")
        NWARM = 20
        for w in range(NWARM):
            nc.tensor.matmul(warm_ps, ones128[0:1, 0:1], ones512,
                             start=(w == 0), stop=(w == NWARM - 1))

        qT_sb = consts.tile([P, CC, NQ], bf16)
        kT_sb = consts.tile([P, CC, M], bf16)
        v_sb = consts.tile([P, MC, NUM_HEADS, HD + 1], bf16)
        xnT_sb = consts.tile([P, CC, M], bf16)
        attn_sb = consts.tile([P, CC, NQ], bf16)

        nc.vector.memset(v_sb[:, :, :, HD:HD + 1], 1.0)

        # ---- building blocks ----
        def conv_chunk(i, late=False):
            ps = psA.tile([P, 512], f32, tag="psA")
            first = True
            for ij in range(4):
                for cc in range(CC):
                    nc.tensor.matmul(
                        ps, xp_sb[:, ij, cc, i * P:(i + 1) * P],
                        wsr_sb[:, ij, cc, :], start=first, stop=False)
                    first = False
            nc.tensor.matmul(ps, ones128, bsr_sb, start=False, stop=True)
            st6 = stats.tile([P, 6], f32, tag="st6")
            nc.vector.bn_stats(out=st6, in_=ps)
            mv = stats.tile([P, 2], f32, tag="mv")
            nc.vector.bn_aggr(out=mv, in_=st6)
            rstd = stats.tile([P, 1], f32, tag="rstd")
            if late:
                # rstd = exp(-0.5*ln(var+eps)): stays in the softmax exp's
                # activation set, so no table reload mid-attention
                lnv = stats.tile([P, 1], f32, tag="lnv")
                nc.scalar.activation(
                    out=lnv, in_=mv[:, 1:2],
                    func=mybir.ActivationFunctionType.Ln, bias=eps_sb,
                    scale=1.0)
                nc.scalar.activation(
                    out=rstd, in_=lnv,
                    func=mybir.ActivationFunctionType.Exp, bias=0.0,
                    scale=-0.5)
            else:
                std = stats.tile([P, 1], f32, tag="std")
                nc.scalar.activation(
                    out=std, in_=mv[:, 1:2],
                    func=mybir.ActivationFunctionType.Sqrt, bias=eps_sb,
                    scale=1.0)
                nc.vector.reciprocal(out=rstd, in_=std)
            xn = xnp.tile([P, 512], bf16, tag="xn")
            nc.vector.tensor_scalar(
                out=xn, in0=ps, scalar1=mv[:, 0:1], scalar2=rstd,
                op0=mybir.AluOpType.subtract, op1=mybir.AluOpType.mult)
            for cc in range(CC):
                nc.scalar.dma_start(
                    out=xnT_sb[:, cc, i * P:(i + 1) * P],
                    in_=xn[:, cc * P:(cc + 1) * P],
                    transpose=True)

        # q projection (channel-major): qT[cq, n]
        def q_proj(n2):
            for cq in range(CC):
                ps = psA.tile([P, 512], f32, tag="psA")
                for cc in range(CC):
                    nc.tensor.matmul(
                        ps, wq_sb[:, cc, cq * P:(cq + 1) * P],
                        xq_sb[:, cc, n2 * 512:(n2 + 1) * 512],
                        start=(cc == 0), stop=(cc == CC - 1))
                nc.vector.tensor_copy(
                    out=qT_sb[:, cq, n2 * 512:(n2 + 1) * 512], in_=ps)

        def v_proj(mc):
            ps = psA.tile([P, 512], f32, tag="psA")
            for cc in range(CC):
                nc.tensor.matmul(
                    ps, xnT_sb[:, cc, mc * P:(mc + 1) * P], wv_sb[:, cc, :],
                    start=(cc == 0), stop=(cc == CC - 1))
            nc.vector.tensor_copy(out=v_sb[:, mc, :, 0:HD], in_=ps)

        def k_proj_m2(m2):
            for ck in range(CC):
                ps = psA.tile([P, 512], f32, tag="psA")
                for cc in range(CC):
                    nc.tensor.matmul(
                        ps, wk_sb[:, cc, ck * P:(ck + 1) * P],
                        xnT_sb[:, cc, m2 * 512:(m2 + 1) * 512],
                        start=(cc == 0), stop=(cc == CC - 1))
                nc.vector.tensor_copy(
                    out=kT_sb[:, ck, m2 * 512:(m2 + 1) * 512], in_=ps)

        def qk_exp(n2, hp, mc_list=None):
            pts = []
            for mc in (mc_list if mc_list is not None else range(MC)):
                qk = psQK.tile([P, 1024], f32, tag="psQK")
                nc.tensor.matmul(
                    qk[:, 0:512],
                    kT_sb[0:HD, hp, mc * P:(mc + 1) * P],
                    qT_sb[0:HD, hp, n2 * 512:(n2 + 1) * 512],
                    start=True, stop=True)
                nc.tensor.matmul(
                    qk[:, 512:1024],
                    kT_sb[HD:P, hp, mc * P:(mc + 1) * P],
                    qT_sb[HD:P, hp, n2 * 512:(n2 + 1) * 512],
                    start=True, stop=True)
                pt = ptp.tile([P, 1024], bf16, tag="pt")
                nc.scalar.activation(
                    out=pt, in_=qk, func=mybir.ActivationFunctionType.Exp)
                pts.append(pt)
            return pts

        def finish_pair(n2, hp, av_e, av_o, last=False):
            # stage av out of PSUM fast (frees the bank for the next pair's
            # attn@v ~2.5us earlier), then normalize entirely on the idle
            # Pool engine (broadcast + multiply, all-SBUF). The final pair
            # skips the staging copy (nothing needs its bank) and multiplies
            # straight from PSUM on the DVE for a shorter drain chain.
            for par, av in ((0, av_e), (1, av_o)):
                if last:
                    rs = stats.tile([1, 512], f32, tag="rs")
                    nc.vector.reciprocal(out=rs, in_=av[HD:HD + 1, :])
                    rsb = stats.tile([HD, 512], f32, tag="rsb")
                    nc.gpsimd.partition_broadcast(rsb, rs)
                    nc.vector.tensor_mul(
                        out=attn_sb[HD * par:HD * (par + 1), hp,
                                    n2 * 512:(n2 + 1) * 512],
                        in0=av[0:HD, :], in1=rsb)
                    continue
                avf = outs.tile([HD + 1, 512], f32, tag="avf")
                nc.vector.tensor_copy(out=avf, in_=av)
                rs = stats.tile([1, 512], f32, tag="rs")
                nc.vector.reciprocal(out=rs, in_=avf[HD:HD + 1, :])
                rsb = stats.tile([HD, 512], f32, tag="rsb")
                nc.gpsimd.partition_broadcast(rsb, rs)
                nc.gpsimd.tensor_tensor(
                    out=attn_sb[HD * par:HD * (par + 1), hp,
                                n2 * 512:(n2 + 1) * 512],
                    in0=avf[0:HD, :], in1=rsb, op=mybir.AluOpType.mult)

        def av_pair(n2, hp, pts, pool=None):
            pool = pool or psAV
            av_e = pool.tile([HD + 1, 512], f32, tag="psA" if pool is psA else "psAV")
            av_o = pool.tile([HD + 1, 512], f32, tag="psA" if pool is psA else "psAV")
            for mc, pt in enumerate(pts):
                nc.tensor.matmul(
                    av_e, v_sb[:, mc, 2 * hp, :], pt[:, 0:512],
                    start=(mc == 0), stop=(mc == MC - 1))
                nc.tensor.matmul(
                    av_o, v_sb[:, mc, 2 * hp + 1, :], pt[:, 512:1024],
                    start=(mc == 0), stop=(mc == MC - 1))
            finish_pair(n2, hp, av_e, av_o)

        def attn_pair(n2, hp, pool=None):
            # qk issued one mc ahead of the attn@v consumer so the exp
            # engine always has a queued tile (avoids PE<->Act ping-pong)
            pool = pool or psAV
            av_e = pool.tile([HD + 1, 512], f32, tag="psA" if pool is psA else "psAV")
            av_o = pool.tile([HD + 1, 512], f32, tag="psA" if pool is psA else "psAV")
            pts = qk_exp(n2, hp, [0])
            for mc in range(MC):
                if mc + 1 < MC:
                    pts += qk_exp(n2, hp, [mc + 1])
                pt = pts[mc]
                nc.tensor.matmul(
                    av_e, v_sb[:, mc, 2 * hp, :], pt[:, 0:512],
                    start=(mc == 0), stop=(mc == MC - 1))
                nc.tensor.matmul(
                    av_o, v_sb[:, mc, 2 * hp + 1, :], pt[:, 512:1024],
                    start=(mc == 0), stop=(mc == MC - 1))
            finish_pair(n2, hp, av_e, av_o)

        def out_proj_start(t):
            # first 3 cc-chunks only need head pairs 0-2: runs in the psAV
            # banks freed after pair (1,2), while pair (1,3) is still in
            # flight, shortening the drain tail
            ps = psAV.tile([P, 512], f32, tag="psAV")
            for cc in range(CC - 1):
                nc.tensor.matmul(
                    ps, attn_sb[:, cc, t * P:(t + 1) * P], wp_sb[:, cc, :],
                    start=(cc == 0), stop=False)
            return ps

        def out_proj_end(t, ps):
            nc.tensor.matmul(
                ps, attn_sb[:, CC - 1, t * P:(t + 1) * P], wp_sb[:, CC - 1, :],
                start=False, stop=False)
            nc.tensor.matmul(ps, ones128, bp_sb, start=False, stop=True)
            ot = outs.tile([P, C], bf16, tag="ot")
            nc.vector.tensor_scalar_mul(
                out=ot, in0=ps, scalar1=mask_sb[:, t:t + 1])
            nc.sync.dma_start(out=out_d[t * P:(t + 1) * P, :], in_=ot)

        def out_proj(t):
            ps = psA.tile([P, 512], f32, tag="psA")
            for cc in range(CC):
                nc.tensor.matmul(
                    ps, attn_sb[:, cc, t * P:(t + 1) * P], wp_sb[:, cc, :],
                    start=(cc == 0), stop=False)
            nc.tensor.matmul(ps, ones128, bp_sb, start=False, stop=True)
            ot = outs.tile([P, C], bf16, tag="ot")
            nc.vector.tensor_scalar_mul(
                out=ot, in0=ps, scalar1=mask_sb[:, t:t + 1])
            nc.sync.dma_start(out=out_d[t * P:(t + 1) * P, :], in_=ot)

        # ---- program order: interleave independent PE work (qproj, v/k
        # proj, early qk) between conv chunks so the PE queue always has
        # ready work when a conv chunk's LN-gated PSUM release fires (the
        # cost model charges a p-state penalty whenever the PE restarts
        # from idle) ----
        conv_chunk(0)
        conv_chunk(1)
        conv_chunk(2)
        q_proj(0)          # xq lands during conv 0-2
        conv_chunk(3)
        q_proj(1)
        conv_chunk(4)
        k_proj_m2(0)       # xnT chunks 0-3
        pts00 = qk_exp(0, 0, [0, 1, 2, 3])   # Act exp starts early
        conv_chunk(5)
        pts01 = qk_exp(0, 1, [0, 1, 2, 3])
        v_proj(0)
        v_proj(1)
        conv_chunk(6)
        pts02 = qk_exp(0, 2, [0, 1, 2, 3])
        v_proj(2)
        v_proj(3)
        conv_chunk(7)
        pts03 = qk_exp(0, 3, [0, 1, 2, 3])
        v_proj(4)          # vproj cover while conv7's LN/transposes drain
        v_proj(5)
        v_proj(6)
        v_proj(7)
        k_proj_m2(1)       # xnT chunks 4-7
        # pre-accumulate pair (0,0)'s attn@v over the early chunks: this
        # consumes (frees) 4 pt-pool buffers before the [4..7] qk allocation
        # crunch that otherwise stalls the exp engine at pair boundaries
        # one-pair-ahead qk issue so the exp engine never idles at pair
        # boundaries
        pts00 += qk_exp(0, 0, [4, 5, 6, 7])
        pts01 += qk_exp(0, 1, [4, 5])
        av_pair(0, 0, pts00)
        pts01 += qk_exp(0, 1, [6, 7])
        pts02 += qk_exp(0, 2, [4, 5])
        av_pair(0, 1, pts01)
        pts02 += qk_exp(0, 2, [6, 7])
        pts03 += qk_exp(0, 3, [4, 5])
        av_pair(0, 2, pts02)
        pts03 += qk_exp(0, 3, [6, 7])
        pts10 = qk_exp(1, 0, [0, 1, 2])
        av_pair(0, 3, pts03)
        # n2=1 pairs: qk issued two mc chunks ahead of the attn@v consumer,
        # rolling into the next pair's first chunks at each pair's end, so
        # the exp engine runs back-to-back across pair boundaries; n2=0
        # out_projs interleave to keep the PE fed. Last pair borrows the
        # idle psA banks to overlap the psAV rotation.
        prefetch = {0: pts10}
        for hp in range(HP):
            pool = psA if hp == HP - 1 else psAV
            tag = "psA" if hp == HP - 1 else "psAV"
            av_e = pool.tile([HD + 1, 512], f32, tag=tag)
            av_o = pool.tile([HD + 1, 512], f32, tag=tag)
            pts = prefetch[hp]
            nxt = []
            for mc in range(MC):
                if len(pts) < MC and len(pts) < mc + 4:
                    pts += qk_exp(1, hp, [len(pts)])
                elif hp + 1 < HP and len(nxt) < 3:
                    nxt += qk_exp(1, hp + 1, [len(nxt)])
                pt = pts[mc]
                nc.tensor.matmul(
                    av_e, v_sb[:, mc, 2 * hp, :], pt[:, 0:512],
                    start=(mc == 0), stop=(mc == MC - 1))
                nc.tensor.matmul(
                    av_o, v_sb[:, mc, 2 * hp + 1, :], pt[:, 512:1024],
                    start=(mc == 0), stop=(mc == MC - 1))
            prefetch[hp + 1] = nxt
            finish_pair(1, hp, av_e, av_o, last=(hp == HP - 1))
            if hp < HP - 1:
                out_proj(hp)
            if hp == 0:
                out_proj(3)   # n2=0 tokens: ready since pair (0,3) finished
        for t in range(4, 8):
            out_proj(t)

    nc.compile()
    return nc


def _prep_inputs(x0, x1, mask0, mask1, Wq, Wkv, Wsr, bsr, gamma, beta, Wp, bp):
    """Host-side layout prep -> per-core in_maps."""
    scale = HD ** (-0.5)
    wq = (Wq * scale).astype(BF16)
    # conv weights: Wsr[co, ci, i, j] -> per (i,j) lhs [ci, co]
    wsr = np.stack([Wsr[:, :, ij // 2, ij % 2].T.copy() for ij in range(4)])
    wsr = wsr.astype(BF16)
    bsr_r = bsr.reshape(1, C).astype(BF16)
    # fold LN gamma/beta into Wkv; drop the k bias (softmax-invariant) and
    # fold the v bias through the output projection: y = attn@Wp + bv@Wp + bp
    Wkv_f = gamma[:, None] * Wkv
    bkv = beta @ Wkv
    wk = Wkv_f[:, :C].astype(BF16)
    wv = Wkv_f[:, C:].astype(BF16)
    bv = bkv[C:]
    wp = Wp.astype(BF16)
    bp_r = (bp + bv @ Wp).reshape(1, C).astype(BF16)

    shared = dict(wq=wq, wsr=wsr, bsr_r=bsr_r, wk=wk, wv=wv, wp=wp, bp_r=bp_r)

    xT = [np.ascontiguousarray(x[0].T).astype(BF16) for x in (x0, x1)]
    # patch-major gather for the conv: xp[ij][c, oh*32+ow] = xT[c, 128*oh+64*i+2*ow+j]
    xp = []
    for b in range(2):
        v = xT[b].reshape(C, 32, 2, 32, 2)
        xp.append(np.stack([
            np.ascontiguousarray(v[:, :, ij // 2, :, ij % 2].reshape(C, M))
            for ij in range(4)]))
    masks = (mask0, mask1)
    in_maps = []
    for core in range(N_CORES):
        b, s = core // 4, core % 4
        m = dict(shared)
        m["xp"] = xp[b]
        m["xqT"] = np.ascontiguousarray(xT[b][:, s * NQ:(s + 1) * NQ])
        msk = masks[b][0, s * NQ:(s + 1) * NQ]
        m["mask_s"] = np.ascontiguousarray(
            msk.reshape(NQ // 128, 128).T).astype(np.float32)
        in_maps.append(m)
    return in_maps


def kernel(x0, x1, mask0, mask1, Wq, Wkv, Wsr, bsr, gamma, beta, Wp, bp,
           H=64, W=64, _trace=False):
    global _compiled
    x0 = np.asarray(x0, np.float32)
    x1 = np.asarray(x1, np.float32)
    mask0 = np.asarray(mask0, np.float32)
    mask1 = np.asarray(mask1, np.float32)
    assert x0.shape == (B, N, C) and int(H) == 64 and int(W) == 64

    from concourse.bass_utils import run_bass_kernel_spmd

    if _compiled is None:
        _compiled = _build()
    nc = _compiled

    in_maps = _prep_inputs(
        x0, x1, mask0, mask1,
        np.asarray(Wq, np.float32), np.asarray(Wkv, np.float32),
        np.asarray(Wsr, np.float32), np.asarray(bsr, np.float32),
        np.asarray(gamma, np.float32), np.asarray(beta, np.float32),
        np.asarray(Wp, np.float32), np.asarray(bp, np.float32))

    kw = {}
    if _trace:
        kw = dict(trace=True, trace_cores=[0])
    try:
        res = run_bass_kernel_spmd(nc, in_maps, list(range(N_CORES)), **kw)
    except ModuleNotFoundError:
        # NTFF profile hook unavailable in this environment -> run untraced
        res = run_bass_kernel_spmd(nc, in_maps, list(range(N_CORES)))

    o0 = np.concatenate(
        [res.results[i]["out"].astype(np.float32) for i in range(4)], axis=0)
    o1 = np.concatenate(
        [res.results[i]["out"].astype(np.float32) for i in range(4, 8)], axis=0)
    keep0 = (mask0[0] >= MASK_THRESHOLD)[:, None]
    keep1 = (mask1[0] >= MASK_THRESHOLD)[:, None]
    y0 = np.where(keep0, o0, o1)[None]
    y1 = np.where(keep1, o1, o0)[None]
    out = np.stack([y0, y1]).astype(np.float32)
    if _trace:
        kernel._last_result = res
    return out


kernel._last_result = None

